# revision 18
# baseline (speedup 1.0000x reference)
"""Causal single-head attention (B=4, S=2048, D=1024, fp32) on 8 TRN2 cores.

Reference computation (per batch b):
    scores = (x @ qk) @ x.T / sqrt(D)   causal-masked, softmax over keys
    out    = softmax(scores) @ x @ ov

Sharding: 2 cores per batch. Each core owns 8 of the 16 128-row query
blocks, snake-assigned ({4k, 4k+3} vs {4k+1, 4k+2}) so both halves see an
identical causal work profile -> one SPMD program, per-core data only.

All matmul operands are bf16 (PSUM accumulation stays fp32; softmax
normalization and the output stay fp32; rms_rel ~4.4e-3 vs the fp32
reference). The host ships x in the layouts the PE wants (natural,
transposed, rows-transposed), so the kernel has no x transposes at all;
the only PE transposes left are the prob tiles (attn^T for the PV
contraction). Work per pair of query blocks is fused: scores -> probs ->
attn^T -> PT = (attn @ x)^T -> out = PT^T @ ov, with pairs processed
heaviest-first so the serial tail after the last score block is the
lightest pair. PT accumulates K-contiguously in PSUM per (pair, d-chunk)
with exact per-column causal extents (N=256 while both blocks are live,
N=128 for the odd-only tail chunks), eliminating SBUF fold traffic.
qT runs dc-outer across all 8 PSUM banks (slots borrowed from the main
pools to avoid a pool-stack barrier); a short dummy-matmul burst fills
the initial DMA wait so the PE's HAM clock gate is warm when real work
arrives. PE occupancy in the cost-model timeline: ~95%.
"""

import numpy as np

import concourse.bacc as bacc
import concourse.mybir as mybir
import concourse.tile as tile
from concourse.bass_interp import get_hw_module
from concourse.bass_utils import run_bass_kernel_spmd

B, S, D = 4, 2048, 1024
NB = S // 128          # 16 row blocks per batch
NBL = NB // 2          # 8 row blocks per core
N_CORES = 8
SCALE = float(np.sqrt(D))
NEG = -1.0e30
DC = D // 128           # 8

# local block -> global block, per half (snake: exactly balanced causal work)
HALF_BLOCKS = [
    [0, 3, 4, 7, 8, 11, 12, 15],
    [1, 2, 5, 6, 9, 10, 13, 14],
]
# 512-wide score strips per local block (same for both halves); block i of
# pair p = i // 2 has CI[i] = p + 1 strips
CI = [1, 1, 2, 2, 3, 3, 4, 4]

F32 = mybir.dt.float32
BF16 = mybir.dt.bfloat16


def _emit(nc, tc, xT_in, xTr_in, xn_in, qk_in, ov_in, masks_in, y_out, ctx):
    const = ctx.enter_context(tc.tile_pool(name="const", bufs=1))

    # thresh input also carries a host-built 128x128 identity (cols NBL..)
    thr_id = const.tile([128, NBL + 128], F32, name="thr_id")
    thresh_sb = thr_id[:, 0:NBL]
    ident16 = const.tile([128, 128], BF16, name="ident16")
    iota_t = const.tile([128, 512], F32, name="iota_t")
    recips = const.tile([128, NBL], F32, name="recips")

    # ---- persistent SBUF data tiles ----
    qk_sb = const.tile([128, DC, 1024], BF16, name="qk_sb")
    ov_sb = const.tile([128, DC, 1024], BF16, name="ov_sb")
    xS = [const.tile([128, DC, 512], BF16, name=f"xS{h}") for h in range(2)]
    qTh = [const.tile([128, DC, 512], BF16, name=f"qTh{h}") for h in range(2)]
    xTs = [const.tile([128, DC, 512], BF16, name=f"xTs{s}") for s in range(4)]
    xn_p = ctx.enter_context(tc.tile_pool(name="xn_p", bufs=NB))
    xns = [xn_p.tile([128, 1024], BF16, name="xn", tag="xn") for _ in range(NB)]
    # attn^T scratch for the current pair: [t-part, t-chunk, 256] where the
    # 256 free cols are [even block rows | odd block rows]
    attnT = const.tile([128, NB, 256], BF16, name="attnT")

    # ---- DMA streams, in consumption order ----
    qk_src = qk_in.rearrange("(c p) e -> p c e", p=128)
    xTr_src = xTr_in.rearrange("(c p) r -> p c r", p=128)
    xT_src = xT_in.rearrange("(c p) t -> p c t", p=128)
    ov_src = ov_in.rearrange("(c p) e -> p c e", p=128)
    # tiny head-of-stream slices so the first qT matmul (stationary
    # qk[:, 0, 0:128], moving xS1[:, 0, :]) unblocks as early as possible
    nc.sync.dma_start(out=qk_sb[:, 0, 0:128], in_=qk_src[:, 0, 0:128])
    nc.sync.dma_start(out=xS[1][:, 0, :], in_=xTr_src[:, 0, 512:1024])
    nc.sync.dma_start(out=qk_sb[:, 0, 128:1024], in_=qk_src[:, 0, 128:1024])
    for dc in range(DC):
        if dc > 0:
            nc.sync.dma_start(out=qk_sb[:, dc, :], in_=qk_src[:, dc, :])
            nc.sync.dma_start(out=xS[1][:, dc, :],
                              in_=xTr_src[:, dc, 512:1024])
        if dc == 0:
            nc.sync.dma_start(out=thr_id, in_=masks_in)
    for dc in range(DC):
        nc.sync.dma_start(out=xS[0][:, dc, :], in_=xTr_src[:, dc, 0:512])
    for st in range(4):
        nc.sync.dma_start(out=xTs[st],
                          in_=xT_src[:, :, st * 512:(st + 1) * 512])
    for tc_i in range(NB):
        nc.sync.dma_start(out=xns[tc_i],
                          in_=xn_in[tc_i * 128:(tc_i + 1) * 128, :])
    for dc in range(DC):
        nc.sync.dma_start(out=ov_sb[:, dc, :], in_=ov_src[:, dc, :])

    # derived constants (emitted after the DMAs so Tile sees the
    # write-before-read on thr_id)
    nc.vector.tensor_copy(ident16, thr_id[:, NBL:NBL + 128])
    # iota 0..511 along free dim; causal mask for block i's last strip is
    # (iota > thresh[:, i]) * NEG with thresh a per-core input
    nc.gpsimd.iota(iota_t, pattern=[[1, 512]], base=0, channel_multiplier=0,
                   allow_small_or_imprecise_dtypes=True)

    psA = ctx.enter_context(tc.tile_pool(name="psA", bufs=3, space="PSUM"))
    psT = ctx.enter_context(tc.tile_pool(name="psT", bufs=2, space="PSUM"))
    psP = ctx.enter_context(tc.tile_pool(name="psP", bufs=3, space="PSUM"))

    # HAM warmup: the PE re-throttles to 1.2 GHz during the inter-rep /
    # startup DMA wait (> 3.4us idle). Burn it with dummy matmuls on a
    # zeroed tile so the real matmuls start at 2.4 GHz.
    dummy_in = const.tile([128, 512], BF16, name="dummy_in")
    nc.vector.memset(dummy_in, 0.0)
    warm_ps = psT.tile([128, 1024], BF16, name="ps_at", tag="psT")
    for k in range(8):
        nc.tensor.matmul(warm_ps.bitcast(F32), dummy_in[:, 0:128], dummy_in,
                         start=(k == 0), stop=(k == 7))

    def ps_any(k):
        # borrow a 1-bank fp32 view from whichever main pool has slot k;
        # lets the qT chains use all 8 banks without a pool-stack boundary
        # (a dedicated pool's release would serialize phase 1's last
        # evacuation against the main loop's first matmul)
        if k < 3:
            return psA.tile([128, 512], F32, name="ps_sc", tag="psA")
        if k < 6:
            return psP.tile([128, 512], F32, name="ps_pt", tag="psP")
        t = psT.tile([128, 1024], BF16, name="ps_at", tag="psT")
        return t.bitcast(F32)

    # ---- phase 1: qT = qk.T @ x_rows.T, half rbg=1 first (the reversed
    # main loop starts with blocks 6,7 which live in half 1). dc-outer so
    # the first matmuls only need the first qk/xS chunks off the wire;
    # 8 concurrent PSUM accumulation chains = all 8 banks. ----
    for rbg in (1, 0):
        chains = [ps_any(k) for k in range(DC)]
        for dc in range(DC):
            for ec in range(DC):
                nc.tensor.matmul(
                    chains[ec], qk_sb[:, dc, ec * 128:(ec + 1) * 128],
                    xS[rbg][:, dc, :],
                    start=(dc == 0), stop=(dc == DC - 1))
        for ec in range(DC):
            # alternate engines so the 8 evacuations drain in ~2.5us
            if ec % 2 == 0:
                nc.scalar.activation(
                    qTh[rbg][:, ec, :], chains[ec],
                    mybir.ActivationFunctionType.Copy)
            else:
                nc.vector.tensor_copy(qTh[rbg][:, ec, :], chains[ec])

    # ---- fused main loop over pairs, heaviest first ----
    pst_p = ctx.enter_context(tc.tile_pool(name="pst_p", bufs=4))
    mask_p = ctx.enter_context(tc.tile_pool(name="mask_p", bufs=2))
    rs_p = ctx.enter_context(tc.tile_pool(name="rs_p", bufs=4))
    PT_p = ctx.enter_context(tc.tile_pool(name="PT_p", bufs=2))
    y_p = ctx.enter_context(tc.tile_pool(name="y_p", bufs=3))

    pend = []

    def flush():
        while pend:
            pend.pop(0)()

    def strip(i, st, rs):
        c = CI[i]
        # even blocks' causal extent ends <=2 chunks into their final
        # strip (snake property, both halves) -> 256-wide work there
        W = 256 if (st == c - 1 and i % 2 == 0) else 512
        nj = W // 128
        ps = psA.tile([128, 512], F32, name="ps_sc", tag="psA")
        for ec in range(DC):
            nc.tensor.matmul(
                ps[:, 0:W],
                qTh[i // 4][:, ec, (i % 4) * 128:(i % 4 + 1) * 128],
                xTs[st][:, ec, 0:W],
                start=(ec == 0), stop=(ec == DC - 1))
        if st == c - 1:
            mask = mask_p.tile([128, 512], F32, name="mask", tag="mask")
            nc.vector.tensor_scalar(
                out=mask[:, 0:W], in0=iota_t[:, 0:W],
                scalar1=thresh_sb[:, i:i + 1], scalar2=NEG,
                op0=mybir.AluOpType.is_gt, op1=mybir.AluOpType.mult)
            nc.vector.tensor_add(ps[:, 0:W], ps[:, 0:W], mask[:, 0:W])
        p_st = pst_p.tile([128, 512], BF16, name="p_st", tag="p_st")
        nc.scalar.activation(
            p_st[:, 0:W], ps[:, 0:W],
            mybir.ActivationFunctionType.Exp,
            scale=1.0 / SCALE, accum_out=rs[:, st:st + 1])

        def post():
            pst2 = psT.tile([128, 1024], BF16, name="ps_at", tag="psT")
            for j in range(nj):
                nc.tensor.transpose(
                    pst2[:, j * 128:(j + 1) * 128],
                    p_st[:, j * 128:(j + 1) * 128], ident16)
            nc.vector.tensor_copy(
                attnT[:, st * 4:st * 4 + nj,
                      (i % 2) * 128:(i % 2) * 128 + 128],
                pst2[:, 0:W].rearrange("p (a b) -> p a b", a=nj))
        pend.append(post)

    def score_block(i):
        rs = rs_p.tile([128, 4], F32, name="rs", tag="rs")
        for st in range(CI[i]):
            strip(i, st, rs)
            if len(pend) > 1:
                pend.pop(0)()
        rsum = rs_p.tile([128, 1], F32, name="rsum", tag="rsum")
        nc.vector.reduce_sum(rsum, rs[:, 0:CI[i]], axis=mybir.AxisListType.X)
        nc.vector.reciprocal(recips[:, i:i + 1], rsum)

    def emit_pair(p, PT_t):
        for bi in range(2):
            i = 2 * p + bi
            y_sb = y_p.tile([128, 1024], F32, name="y_sb", tag="y_sb")
            for es in range(2):
                ps = psA.tile([128, 512], F32, name="ps_o", tag="psA")
                for dc in range(DC):
                    nc.tensor.matmul(
                        ps, PT_t[:, dc, bi * 128:(bi + 1) * 128],
                        ov_sb[:, dc, es * 512:(es + 1) * 512],
                        start=(dc == 0), stop=(dc == DC - 1))
                # alternate evacuation engines and stream each half out as
                # soon as it's scaled, to shorten the serial tail
                if es == 0:
                    nc.scalar.activation(
                        y_sb[:, 0:512], ps,
                        mybir.ActivationFunctionType.Copy,
                        scale=recips[:, i:i + 1])
                else:
                    nc.vector.tensor_scalar(
                        out=y_sb[:, 512:1024], in0=ps,
                        scalar1=recips[:, i:i + 1], scalar2=None,
                        op0=mybir.AluOpType.mult)
                nc.sync.dma_start(
                    out=y_out[i * 128:(i + 1) * 128, es * 512:(es + 1) * 512],
                    in_=y_sb[:, es * 512:(es + 1) * 512])

    prev = None
    for p in (3, 2, 1, 0):
        score_block(2 * p)
        score_block(2 * p + 1)
        flush()
        if prev is not None:
            emit_pair(*prev)
        # PT = (attn @ x)^T for this pair, K-contiguous over t-chunks.
        # The even block's causal extent ends at chunk 4p+2 (both halves),
        # so its half of the 256 prob columns drops out of the last two
        # chunks: narrow those matmuls to the odd block's 128 columns.
        PT_t = PT_p.tile([128, DC, 256], BF16, name="PT_t", tag="PT")
        E = 4 * (p + 1)
        Ee = 4 * p + 2
        for dc in range(DC):
            ps = psP.tile([128, 512], F32, name="ps_pt", tag="psP")
            for t in range(E):
                lo = 0 if t < Ee else 128
                nc.tensor.matmul(
                    ps[:, lo:256],
                    xns[t][:, dc * 128:(dc + 1) * 128],
                    attnT[:, t, lo:256],
                    start=(t == 0), stop=(t == E - 1),
                    skip_group_check=True)
            nc.vector.tensor_copy(PT_t[:, dc, :], ps[:, 0:256])
        prev = (p, PT_t)
    emit_pair(*prev)


_BUILT = {}


def _build(n_reps=1, timing=False, hw=True):
    """timing=True builds a variant whose big tensors are Internal DRAM
    (garbage data, tiny external IO) so per-call transfer overhead over the
    axon tunnel doesn't swamp wall-clock differencing."""
    key = (n_reps, timing, hw)
    if key in _BUILT:
        return _BUILT[key]
    from contextlib import ExitStack

    nc = bacc.Bacc(
        "TRN2", target_bir_lowering=False, debug=False,
        enable_asserts=False, num_devices=N_CORES)
    big = dict(kind="Internal") if timing else {}
    xT_in = nc.dram_tensor("xT", [D, S], BF16,
                           **(big or dict(kind="ExternalInput"))).ap()
    xTr_in = nc.dram_tensor("xTr", [D, S // 2], BF16,
                            **(big or dict(kind="ExternalInput"))).ap()
    xn_in = nc.dram_tensor("xn", [S, D], BF16,
                           **(big or dict(kind="ExternalInput"))).ap()
    qk_in = nc.dram_tensor("qk", [D, D], BF16,
                           **(big or dict(kind="ExternalInput"))).ap()
    ov_in = nc.dram_tensor("ov", [D, D], BF16,
                           **(big or dict(kind="ExternalInput"))).ap()
    masks_in = nc.dram_tensor(
        "thresh", [128, NBL + 128], F32, kind="ExternalInput").ap()
    y_out = nc.dram_tensor("y", [S // 2, D], F32,
                           **(big or dict(kind="ExternalOutput"))).ap()
    dummy_out = None
    if timing:
        dummy_out = nc.dram_tensor(
            "dummy_y", [128, 128], F32, kind="ExternalOutput").ap()

    with tile.TileContext(nc) as tc:
        if timing and n_reps > 1:
            with tc.For_i(0, n_reps, 1):
                with ExitStack() as ctx:
                    _emit(nc, tc, xT_in, xTr_in, xn_in, qk_in, ov_in,
                          masks_in, y_out, ctx)
        else:
            for _ in range(n_reps):
                with ExitStack() as ctx:
                    _emit(nc, tc, xT_in, xTr_in, xn_in, qk_in, ov_in,
                          masks_in, y_out, ctx)
        if timing:
            with tc.tile_pool(name="dummy_p", bufs=1) as dp:
                dt_ = dp.tile([128, 128], F32, name="dummy_sb")
                nc.sync.dma_start(out=dt_, in_=y_out[0:128, 0:128])
                nc.sync.dma_start(out=dummy_out, in_=dt_)
    nc.compile()
    if hw:
        nc.m = get_hw_module(nc.m)
    _BUILT[key] = nc
    return nc


def host_thresh():
    """Columns 0..NBL-1: thresh[r, i] such that last-strip column tcol is
    causally valid for row r of local block i iff tcol <= thresh[r, i].
    Columns NBL..: a 128x128 identity (stationary operand for the PE
    transposes, shipped from host to keep gpsimd off the critical path)."""
    th = np.zeros((2, 128, NBL + 128), np.float32)
    for half in range(2):
        for i, g in enumerate(HALF_BLOCKS[half]):
            th[half, :, i] = 128 * g + np.arange(128) - 512 * (CI[i] - 1)
        th[half, :, NBL:] = np.eye(128, dtype=np.float32)
    return th


def make_in_maps(input_data, qk, ov):
    import ml_dtypes
    BF = ml_dtypes.bfloat16
    x = np.asarray(input_data, dtype=np.float32)
    qk16 = np.ascontiguousarray(np.asarray(qk, dtype=np.float32).astype(BF))
    ov16 = np.ascontiguousarray(np.asarray(ov, dtype=np.float32).astype(BF))
    th = host_thresh()
    in_maps = []
    for c in range(N_CORES):
        b, half = c // 2, c % 2
        xb16 = x[b].astype(BF)
        rows = np.concatenate(
            [xb16[128 * g:128 * (g + 1), :] for g in HALF_BLOCKS[half]],
            axis=0)
        in_maps.append({
            "xT": np.ascontiguousarray(xb16.T),
            "xTr": np.ascontiguousarray(rows.T),
            "xn": np.ascontiguousarray(xb16),
            "qk": qk16,
            "ov": ov16,
            "thresh": np.ascontiguousarray(th[half]),
        })
    return in_maps


def assemble(results):
    out = np.empty((B, S, D), np.float32)
    for c in range(N_CORES):
        b, half = c // 2, c % 2
        y = results[c]["y"]
        for i, g in enumerate(HALF_BLOCKS[half]):
            out[b, 128 * g:128 * (g + 1), :] = y[128 * i:128 * (i + 1), :]
    return out


def kernel(input_data, qk, ov):
    nc = _build()
    in_maps = make_in_maps(input_data, qk, ov)
    res = run_bass_kernel_spmd(nc, in_maps, core_ids=list(range(N_CORES)))
    return assemble(res.results)
